# revision 31
# baseline (speedup 1.0000x reference)
"""Trainium2 Bass kernel for nn_DownSampler (SelectTopK pooling + linear + KNN graph).

Contract: kernel(**inputs) takes FULL inputs (x [B*N,128], pos [B*N,3], batch [B*N],
w_sel [128], w_lin [128,128], b_lin [128]) and returns the full
(x_c, pos_c, batch_c, edge_index) tuple, matching the jax reference.

Sharding: graph-level data parallelism — 16 graphs across 8 NeuronCores,
2 graphs per core, weights replicated.

Selection fidelity: the reference's score pipeline is `(x @ w_sel) /
jnp.linalg.norm(w_sel)` (host numpy matmul when called with numpy inputs)
followed by a device softmax and `jax.lax.top_k`. Top-k ordering decides the
row order of every output, and near-ties flip under any numerical deviation,
so kernel() reproduces that exact pipeline (same ops, same platform) and
derives the permutation with a stable argsort (verified bit-equal to the
device top_k). All heavy per-graph work runs in the Bass SPMD kernel:
  KNN top-6 over the kept 4096 nodes per graph:
      candidate metric -|p_j|^2/2 + p_i.p_j via K=4 PE matmul (rank-ordering
      equal to d2 along each row), DVE Max8/MaxIndex top-8 candidates,
      exact (p_i-p_j)^2 refinement (bit-matching the reference's d2 formula)
      with a stable (d2, idx) sort of 8,
  x_c = x[kept] @ w_lin.T + b_lin  (indirect row gathers + PE matmul),
  pos_c gather.
Host assembles batch_c (gather) and edge_index (pure index arithmetic).
"""

import sys

import numpy as np

if "/opt/trn_rl_repo" not in sys.path:
    sys.path.insert(0, "/opt/trn_rl_repo")

B, N, D, NCK, KNN = 16, 16384, 128, 4096, 6
GPC = 2  # graphs per core
M = 8  # cores
BIGF = 3.0e38

_cache = {}


def _build_program():
    import concourse.bass as bass
    import concourse.bacc as bacc
    import concourse.mybir as mybir
    import concourse.tile as tile
    from concourse.masks import make_identity

    f32 = mybir.dt.float32
    i32 = mybir.dt.int32
    u32 = mybir.dt.uint32
    Alu = mybir.AluOpType
    CPY = mybir.ActivationFunctionType.Copy

    nc = bacc.Bacc(None, target_bir_lowering=False, debug=True)

    xg = nc.dram_tensor("xg", [GPC * N, D], f32, kind="ExternalInput")
    posg = nc.dram_tensor("posg", [GPC * N, 3], f32, kind="ExternalInput")
    idxg = nc.dram_tensor("idxg", [GPC, NCK], i32, kind="ExternalInput")
    wlt = nc.dram_tensor("wlt", [D, D], f32, kind="ExternalInput")
    bb = nc.dram_tensor("bb", [128, D], f32, kind="ExternalInput")

    xc_out = nc.dram_tensor("xc_out", [GPC * NCK, D], f32, kind="ExternalOutput")
    pc_out = nc.dram_tensor("pc_out", [GPC, NCK, 3], f32, kind="ExternalOutput")
    nbr_out = nc.dram_tensor("nbr_out", [GPC, NCK, KNN], i32, kind="ExternalOutput")

    # ---- host constants (baked into the NEFF) ----
    parr = np.arange(128)
    farr = np.arange(128)
    # FB rows k: bit k of f — used as bitonic direction rows for the sort-of-8
    fb_np = np.concatenate(
        [
            np.tile(((farr >> k) & 1).astype(np.float32), (128, 2))
            for k in range(8)
        ],
        axis=1,
    )  # [128, 2048]
    iotaT_np = (
        np.arange(32)[None, :] * 128 + parr[:, None]
    ).astype(np.float32)  # [128,32]: rank of (p, rc)

    fb_d = nc.inline_tensor(np.ascontiguousarray(fb_np), name="c_fb")
    iotaT_d = nc.inline_tensor(np.ascontiguousarray(iotaT_np), name="c_iotaT")
    ones_d = nc.inline_tensor(np.ones((1, NCK), np.float32), name="c_ones")

    import contextlib

    with tile.TileContext(nc) as tc, contextlib.ExitStack() as es:
        persist = es.enter_context(tc.tile_pool(name="persist", bufs=1))
        tpsum = es.enter_context(tc.tile_pool(name="tpsum", bufs=2, space="PSUM"))
        scratch = es.enter_context(tc.tile_pool(name="scratch", bufs=2))

        ident = persist.tile([128, 128], f32)
        make_identity(nc, ident[:])

        wlt_sb = persist.tile([128, 128], f32)
        nc.sync.dma_start(wlt_sb[:], wlt[:])
        bb_sb = persist.tile([128, 128], f32)
        nc.sync.dma_start(bb_sb[:], bb[:])
        fb_sb = persist.tile([128, 2048], f32)
        nc.sync.dma_start(fb_sb[:], fb_d[:])
        iotaT_sb = persist.tile([128, 32], f32)
        nc.sync.dma_start(iotaT_sb[:], iotaT_d[:])
        bigt = persist.tile([128, 8], f32)
        nc.vector.memset(bigt[:], BIGF)
        ones3 = persist.tile([3, 1], f32)
        nc.vector.memset(ones3[:], 1.0)

        # ============ Phase C: kept ids, pos gathers, aug matrices ============
        B4s, A4s, pks, O2gs = [], [], [], []
        with tc.tile_pool(name="np2", bufs=2, space="PSUM") as np2p, \
             tc.tile_pool(name="cmisc", bufs=2) as cmisc:
            for g in range(GPC):
                # O2g[p, rc] = global row id (within this core) of rank rc*128+p
                O2 = persist.tile([128, 32], i32, tag=f"O2_{g}")
                nc.sync.dma_start(
                    O2[:], idxg[g, :].rearrange("(rc p) -> p rc", p=128)
                )
                O2g = persist.tile([128, 32], i32, tag=f"O2g_{g}")
                nc.vector.tensor_scalar(
                    O2g[:], O2[:], g * N, None, op0=Alu.add
                )
                O2gs.append(O2g)
                # pos gather: one row-gather per rank-chunk (HW indirect DMA
                # consumes exactly one offset per partition).
                # pk[p, rc*3+c] = pos[rank rc*128+p][c]
                pk = persist.tile([128, 96], f32, tag=f"pk_{g}")
                for rc in range(32):
                    nc.gpsimd.indirect_dma_start(
                        out=pk[:, rc * 3 : (rc + 1) * 3],
                        out_offset=None,
                        in_=posg[:],
                        in_offset=bass.IndirectOffsetOnAxis(
                            ap=O2g[:, rc : rc + 1], axis=0
                        ),
                    )
                pks.append(pk)
                nc.sync.dma_start(
                    pc_out[g].rearrange("(rc p) c -> p rc c", p=128),
                    pk[:].rearrange("p (rc c) -> p rc c", c=3),
                )
                # Candidate metric: met'[i,j] = -|p_j|^2/2 + p_i . p_j
                # (monotone-decreasing in d2 along each row; max8 desc = d2 asc)
                # A4 = [1; x_i; y_i; z_i]   B4 = [-|p_j|^2/2; x_j; y_j; z_j]
                # C3[c, rank] built by PE-transposing the per-chunk pos tiles.
                C3 = cmisc.tile([3, NCK], f32, tag="C3")
                for rc in range(32):
                    p3 = tpsum.tile([128, 128], f32, tag="xtp")
                    nc.tensor.transpose(
                        out=p3[0:3, :],
                        in_=pk[:, rc * 3 : (rc + 1) * 3],
                        identity=ident[:],
                    )
                    nc.scalar.activation(
                        C3[:, rc * 128 : (rc + 1) * 128], p3[0:3, :], CPY
                    )
                B4sq = cmisc.tile([3, NCK], f32, tag="B4sq")
                nc.vector.tensor_tensor(B4sq[:], C3[:], C3[:], op=Alu.mult)
                B4 = persist.tile([4, NCK], f32, tag=f"B4_{g}")
                for q in range(8):
                    qp = np2p.tile([1, 512], f32)
                    nc.tensor.matmul(
                        out=qp[:],
                        lhsT=ones3[:],
                        rhs=B4sq[:, q * 512 : (q + 1) * 512],
                        start=True,
                        stop=True,
                    )
                    nc.scalar.activation(
                        B4[0:1, q * 512 : (q + 1) * 512], qp[:], CPY, scale=-0.5
                    )
                nc.sync.dma_start(B4[1:4, :], C3[:])
                A4 = persist.tile([4, NCK], f32, tag=f"A4_{g}")
                nc.sync.dma_start(A4[0:1, :], ones_d[:])
                nc.sync.dma_start(A4[1:4, :], C3[:])
                B4s.append(B4)
                A4s.append(A4)

        # ============ Phase D: KNN (select + refine)  |  Phase E: linear ======
        with tc.tile_pool(name="d2psum", bufs=1, space="PSUM") as d2psum, \
             tc.tile_pool(name="linpsum", bufs=2, space="PSUM") as linpsum, \
             tc.tile_pool(name="sd2p", bufs=2) as sd2p, \
             tc.tile_pool(name="small", bufs=3) as small, \
             tc.tile_pool(name="linp", bufs=3) as linp:
            for g in range(GPC):
                A4, B4, pk, O2g = A4s[g], B4s[g], pks[g], O2gs[g]
                R = persist.tile([128, 256], f32, tag=f"R_{g}")
                RI = persist.tile([128, 256], f32, tag=f"RI_{g}")
                for rc in range(32):
                    sd2 = sd2p.tile([128, NCK], f32)
                    for half in range(2):
                        dp = d2psum.tile([128, 2048], f32)
                        for q in range(4):
                            c0 = half * 2048 + q * 512
                            nc.tensor.matmul(
                                out=dp[:, q * 512 : (q + 1) * 512],
                                lhsT=A4[:, rc * 128 : (rc + 1) * 128],
                                rhs=B4[:, c0 : c0 + 512],
                                start=True,
                                stop=True,
                            )
                        nc.scalar.activation(
                            sd2[:, half * 2048 : (half + 1) * 2048], dp[:], CPY
                        )
                    mx = small.tile([128, 8], f32, tag="mx")
                    nc.vector.max(mx[:], sd2[:])
                    mi = small.tile([128, 8], u32, tag="mi")
                    nc.vector.max_index(mi[:], mx[:], sd2[:])
                    mii = small.tile([128, 8], i32, tag="mii")
                    nc.vector.tensor_copy(mii[:], mi[:])
                    mif = small.tile([128, 8], f32, tag="mif")
                    nc.vector.tensor_copy(mif[:], mi[:])
                    # exact d2 refinement: gather candidate coords (one
                    # row-gather per candidate slot), diff-square-sum in the
                    # same fp32 association order as the reference
                    G = small.tile([128, 24], f32, tag="G")
                    for j in range(8):
                        nc.gpsimd.indirect_dma_start(
                            out=G[:, j * 3 : (j + 1) * 3],
                            out_offset=None,
                            in_=pc_out[:].rearrange("g r c -> (g r) c"),
                            in_offset=bass.IndirectOffsetOnAxis(
                                ap=mii[:, j : j + 1], axis=0
                            ),
                            element_offset=g * NCK * 3,
                        )
                    Gv = G[:].rearrange("p (j c) -> p c j", c=3)
                    sqs = []
                    for c in range(3):
                        dc = small.tile([128, 8], f32, tag=f"dc{c}")
                        nc.vector.tensor_scalar(
                            dc[:],
                            Gv[:, c, :],
                            pk[:, rc * 3 + c : rc * 3 + c + 1],
                            None,
                            op0=Alu.subtract,
                        )
                        sq = small.tile([128, 8], f32, tag=f"sq{c}")
                        nc.vector.tensor_tensor(sq[:], dc[:], dc[:], op=Alu.mult)
                        sqs.append(sq)
                    s01 = small.tile([128, 8], f32, tag="s01")
                    nc.vector.tensor_tensor(s01[:], sqs[0][:], sqs[1][:], op=Alu.add)
                    rsl = R[:, rc * 8 : (rc + 1) * 8]
                    nc.vector.tensor_tensor(rsl, s01[:], sqs[2][:], op=Alu.add)
                    ms = small.tile([128, 8], i32, tag="ms")
                    nc.vector.tensor_scalar(
                        ms[:],
                        mif[:],
                        iotaT_sb[:, rc : rc + 1],
                        None,
                        op0=Alu.is_equal,
                    )
                    nc.vector.copy_predicated(rsl, ms[:], bigt[:])
                    nc.vector.tensor_copy(RI[:, rc * 8 : (rc + 1) * 8], mif[:])

                # stable sort-of-8 segments, ascending by (d2, idx)
                def seg8(ap, s):
                    v = ap.rearrange(
                        "p (seg blk t e) -> p seg blk t e", seg=32, t=2, e=s
                    )
                    return v[:, :, :, 0, :], v[:, :, :, 1, :]

                for kk8 in range(1, 4):
                    for jj8 in range(kk8 - 1, -1, -1):
                        s = 1 << jj8
                        av, bv = seg8(R[:], s)
                        ai, bi = seg8(RI[:], s)
                        gtv = scratch.tile([128, 256], f32, tag="lt")
                        eqv = scratch.tile([128, 256], f32, tag="m")
                        gti = scratch.tile([128, 256], f32, tag="tv")
                        m8 = scratch.tile([128, 256], i32, tag="mi32")
                        t8v = scratch.tile([128, 256], f32, tag="t8v")
                        t8i = scratch.tile([128, 256], f32, tag="t8i")
                        gtvv = seg8(gtv[:], s)[0]
                        eqvv = seg8(eqv[:], s)[0]
                        gtiv = seg8(gti[:], s)[0]
                        m8v = seg8(m8[:], s)[0]
                        t8vv = seg8(t8v[:], s)[0]
                        t8iv = seg8(t8i[:], s)[0]
                        nc.vector.tensor_tensor(gtvv, av, bv, op=Alu.is_gt)
                        nc.vector.tensor_tensor(eqvv, av, bv, op=Alu.is_equal)
                        nc.vector.tensor_tensor(gtiv, ai, bi, op=Alu.is_gt)
                        nc.vector.tensor_tensor(eqvv, eqvv, gtiv, op=Alu.mult)
                        nc.vector.tensor_tensor(gtvv, gtvv, eqvv, op=Alu.add)
                        krow8 = kk8 if kk8 < 3 else 7  # row 7 all-zero (asc)
                        fbv = seg8(fb_sb[:, krow8 * 256 : (krow8 + 1) * 256], s)[0]
                        nc.vector.tensor_tensor(m8v, gtvv, fbv, op=Alu.subtract)
                        nc.vector.tensor_copy(t8vv, av)
                        nc.vector.copy_predicated(av, m8v, bv)
                        nc.vector.copy_predicated(bv, m8v, t8vv)
                        nc.vector.tensor_copy(t8iv, ai)
                        nc.vector.copy_predicated(ai, m8v, bi)
                        nc.vector.copy_predicated(bi, m8v, t8iv)

                RIi = persist.tile([128, 192], i32, tag=f"RIi_{g}")
                nc.vector.tensor_copy(
                    RIi[:].rearrange("p (seg k) -> p seg k", k=6),
                    RI[:].rearrange("p (seg k) -> p seg k", k=8)[:, :, 0:6],
                )
                nc.sync.dma_start(
                    nbr_out[g].rearrange("(rc p) k -> p rc k", p=128),
                    RIi[:].rearrange("p (rc k) -> p rc k", k=6),
                )

                # -------- Phase E: gather + linear --------
                for rc in range(32):
                    xs = linp.tile([128, 128], f32, tag="xs")
                    nc.gpsimd.indirect_dma_start(
                        out=xs[:],
                        out_offset=None,
                        in_=xg[:],
                        in_offset=bass.IndirectOffsetOnAxis(
                            ap=O2g[:, rc : rc + 1], axis=0
                        ),
                    )
                    xtp2 = tpsum.tile([128, 128], f32, tag="xtp")
                    nc.tensor.transpose(out=xtp2[:], in_=xs[:], identity=ident[:])
                    xts2 = linp.tile([128, 128], f32, tag="xts2")
                    nc.scalar.activation(xts2[:], xtp2[:], CPY)
                    lp = linpsum.tile([128, 128], f32)
                    nc.tensor.matmul(
                        out=lp[:], lhsT=xts2[:], rhs=wlt_sb[:], start=True, stop=True
                    )
                    xo = linp.tile([128, 128], f32, tag="xo")
                    nc.vector.tensor_tensor(xo[:], lp[:], bb_sb[:], op=Alu.add)
                    r0 = g * NCK + rc * 128
                    nc.sync.dma_start(xc_out[r0 : r0 + 128, :], xo[:])

    return nc


def _get_program():
    if "nc" not in _cache:
        nc = _build_program()
        if not nc.is_finalized():
            nc.finalize()  # run bacc passes (reg alloc, wait splitting)
        _cache["nc"] = nc
    return _cache["nc"]


def _select_topk(x, w_sel):
    """Replicate the reference's selection pipeline bit-exactly.

    Mirrors: score = (x @ w_sel) / jnp.linalg.norm(w_sel)  (numpy matmul on
    host when inputs are numpy, device otherwise — same as the reference
    receives), softmax on device, and top-k with lowest-index tie-breaking
    (jax.lax.top_k ≡ stable argsort on the softmax values).
    Returns idx [B, NCK] int32.
    """
    import jax
    import jax.numpy as jnp

    score = (x @ w_sel) / jnp.linalg.norm(w_sel)
    v = jax.nn.softmax(jnp.asarray(score).reshape(B, N), axis=-1)
    vn = np.asarray(v)
    idx = np.argsort(-vn, axis=1, kind="stable")[:, :NCK]
    return np.ascontiguousarray(idx.astype(np.int32))


def _make_in_maps(x, pos, idx, w_lin, b_lin):
    x = np.ascontiguousarray(np.asarray(x, dtype=np.float32))
    pos = np.ascontiguousarray(np.asarray(pos, dtype=np.float32))
    wlt = np.ascontiguousarray(np.asarray(w_lin, dtype=np.float32).T)
    bb = np.ascontiguousarray(
        np.tile(np.asarray(b_lin, dtype=np.float32).reshape(1, D), (128, 1))
    )
    in_maps = []
    for c in range(M):
        r0 = c * GPC * N
        in_maps.append(
            {
                "xg": x[r0 : r0 + GPC * N],
                "posg": pos[r0 : r0 + GPC * N],
                "idxg": idx[c * GPC : (c + 1) * GPC],
                "wlt": wlt,
                "bb": bb,
            }
        )
    return in_maps


def _assemble(results, idx, batch):
    x_c = np.concatenate([r["xc_out"] for r in results], axis=0)  # [B*NCK, D]
    pos_c = np.concatenate(
        [r["pc_out"].reshape(GPC * NCK, 3) for r in results], axis=0
    )
    nbr = np.concatenate([r["nbr_out"] for r in results], axis=0)  # [B, NCK, KNN]

    perm = (
        idx.astype(np.int64) + (np.arange(B, dtype=np.int64) * N)[:, None]
    ).reshape(-1)
    batch_c = np.asarray(batch)[perm]

    off = (np.arange(B, dtype=np.int32) * NCK)[:, None, None]
    src = (nbr.astype(np.int32) + off).reshape(-1)
    ctr = np.broadcast_to(
        np.arange(NCK, dtype=np.int32)[None, :, None] + off, (B, NCK, KNN)
    ).reshape(-1)
    edge_index = np.stack(
        [np.concatenate([src, ctr]), np.concatenate([ctr, src])]
    ).astype(np.int32)
    return x_c, pos_c, batch_c, edge_index


def kernel(x, pos, batch, w_sel, w_lin, b_lin):
    from concourse.bass_utils import run_bass_kernel_spmd

    nc = _get_program()
    idx = _select_topk(x, w_sel)
    in_maps = _make_in_maps(x, pos, idx, w_lin, b_lin)
    res = run_bass_kernel_spmd(nc, in_maps, list(range(M)))
    return _assemble(res.results, idx, batch)


# revision 43
# speedup vs baseline: 6.3719x; 6.3719x over previous
"""Trainium2 Bass kernel for nn_DownSampler (SelectTopK pooling + linear + KNN graph).

Contract: kernel(**inputs) takes FULL inputs (x [B*N,128], pos [B*N,3], batch [B*N],
w_sel [128], w_lin [128,128], b_lin [128]) and returns the full
(x_c, pos_c, batch_c, edge_index) tuple, matching the jax reference.

Sharding: graph-level data parallelism — 16 graphs across 8 NeuronCores,
2 graphs per core, weights replicated.

Selection fidelity: the reference's score pipeline is `(x @ w_sel) /
jnp.linalg.norm(w_sel)` (host numpy matmul when called with numpy inputs)
followed by a device softmax and `jax.lax.top_k`. Top-k ordering decides the
row order of every output, and near-ties flip under any numerical deviation,
so kernel() reproduces that exact pipeline (same ops, same platform) and
derives the permutation with a stable argsort (verified bit-equal to the
device top_k). The heavy per-graph work runs in the Bass SPMD kernel:
  KNN top-6 over the kept 4096 nodes per graph (the dominant compute):
      candidate metric -|p_j|^2/2 + p_i.p_j via K=4 PE matmul (row-wise
      rank-equal to d2), DVE Max8/MaxIndex top-8 candidates per node,
      exact (p_i-p_j)^2 refinement (bit-matching the reference's d2 formula)
      with a stable (d2, idx) bitonic sort of 8,
  x_c = x[kept] @ w_lin.T + b_lin  (PE matmul per 128-row tile).
Host assembles batch_c (gather) and edge_index (pure index arithmetic).
"""

import sys

import numpy as np

if "/opt/trn_rl_repo" not in sys.path:
    sys.path.insert(0, "/opt/trn_rl_repo")

B, N, D, NCK, KNN = 16, 16384, 128, 4096, 6
GPC = 2  # graphs per core
M = 8  # cores
BIGF = 3.0e38

_cache = {}
_SKIP = set()  # debug: subset of {"E","sort8","refine","gather","scan","d2"}


def _build_program():
    import concourse.bass as bass
    import concourse.bacc as bacc
    import concourse.mybir as mybir
    import concourse.tile as tile
    from concourse.masks import make_identity

    f32 = mybir.dt.float32
    i32 = mybir.dt.int32
    u32 = mybir.dt.uint32
    Alu = mybir.AluOpType
    CPY = mybir.ActivationFunctionType.Copy

    nc = bacc.Bacc(None, target_bir_lowering=False, debug=True)

    # host pre-gathered per-core inputs (row order = kept rank order)
    xsel = nc.dram_tensor("xsel", [GPC * NCK, D], f32, kind="ExternalInput")
    psel = nc.dram_tensor("psel", [GPC, NCK, 3], f32, kind="ExternalInput")
    wlt = nc.dram_tensor("wlt", [D, D], f32, kind="ExternalInput")
    bb = nc.dram_tensor("bb", [1, D], f32, kind="ExternalInput")

    xc_out = nc.dram_tensor("xc_out", [GPC * NCK, D], f32, kind="ExternalOutput")
    pc_out = nc.dram_tensor("pc_out", [GPC, NCK, 3], f32, kind="ExternalOutput")
    nbr_out = nc.dram_tensor("nbr_out", [GPC, NCK, KNN], i32, kind="ExternalOutput")

    # ---- host constants (baked into the NEFF) ----
    farr = np.arange(128)
    # FB rows k: bit k of f — bitonic direction rows for the sort-of-8
    fb_np = np.concatenate(
        [np.tile(((farr >> k) & 1).astype(np.float32), (128, 2)) for k in range(8)],
        axis=1,
    )  # [128, 2048]
    # rankof[p, rc*8+j] = rank id rc*128+p (for self-candidate masking)
    rankof_np = (
        np.repeat(np.arange(32)[None, :] * 128, 8, axis=1).reshape(1, 256)
        + np.arange(128)[:, None]
    ).astype(np.float32)

    fb_d = nc.inline_tensor(np.ascontiguousarray(fb_np), name="c_fb")
    rankof_d = nc.inline_tensor(np.ascontiguousarray(rankof_np), name="c_rankof")
    ones_d = nc.inline_tensor(np.ones((1, NCK), np.float32), name="c_ones")

    pselflat = psel[:].rearrange("g r c -> (g r) c")

    import contextlib

    with tile.TileContext(nc) as tc, contextlib.ExitStack() as es:
        persist = es.enter_context(tc.tile_pool(name="persist", bufs=1))
        tpsum = es.enter_context(tc.tile_pool(name="tpsum", bufs=2, space="PSUM"))
        scratch = es.enter_context(tc.tile_pool(name="scratch", bufs=2))

        ident = persist.tile([128, 128], f32)
        make_identity(nc, ident[:])

        wlt_sb = persist.tile([128, 128], f32)
        nc.sync.dma_start(wlt_sb[:], wlt[:])
        bb_sb = persist.tile([1, 128], f32)
        nc.sync.dma_start(bb_sb[:], bb[:])
        ones1 = persist.tile([1, 128], f32)
        nc.vector.memset(ones1[:], 1.0)
        fb_sb = persist.tile([128, 2048], f32)
        nc.sync.dma_start(fb_sb[:], fb_d[:])
        rankof_sb = persist.tile([128, 256], f32)
        nc.sync.dma_start(rankof_sb[:], rankof_d[:])
        bigw = persist.tile([128, 256], f32)
        nc.vector.memset(bigw[:], BIGF)
        ones3 = persist.tile([3, 1], f32)
        nc.vector.memset(ones3[:], 1.0)

        # pos_c is a pure passthrough of the host-gathered kept positions
        nc.sync.dma_start(pc_out[:], psel[:])

        # ============ Phase C: kept positions, aug matrices ============
        B4s, A4s, pks = [], [], []
        with tc.tile_pool(name="np2", bufs=2, space="PSUM") as np2p, \
             tc.tile_pool(name="cmisc", bufs=2) as cmisc:
            for g in range(GPC):
                # pk[p, rc*3+c] = pos of rank rc*128+p, coord c
                pk = persist.tile([128, 96], f32, tag=f"pk_{g}")
                nc.sync.dma_start(
                    pk[:].rearrange("p (rc c) -> p rc c", c=3),
                    psel[g].rearrange("(rc p) c -> p rc c", p=128),
                )
                pks.append(pk)
                # Candidate metric: met'[i,j] = -|p_j|^2/2 + p_i . p_j
                # (row-wise rank-equal to -d2; max8 desc = d2 asc)
                # A4 = [1; x_i; y_i; z_i]   B4 = [-|p_j|^2/2; x_j; y_j; z_j]
                # C3[c, rank] built by PE-transposing the per-chunk pos tiles.
                C3 = cmisc.tile([3, NCK], f32, tag="C3")
                for rc in range(32):
                    p3 = tpsum.tile([128, 128], f32, tag="xtp")
                    nc.tensor.transpose(
                        out=p3[0:3, :],
                        in_=pk[:, rc * 3 : (rc + 1) * 3],
                        identity=ident[:],
                    )
                    nc.scalar.activation(
                        C3[:, rc * 128 : (rc + 1) * 128], p3[0:3, :], CPY
                    )
                B4sq = cmisc.tile([3, NCK], f32, tag="B4sq")
                nc.vector.tensor_tensor(B4sq[:], C3[:], C3[:], op=Alu.mult)
                B4 = persist.tile([4, NCK], f32, tag=f"B4_{g}")
                for q in range(8):
                    qp = np2p.tile([1, 512], f32)
                    nc.tensor.matmul(
                        out=qp[:],
                        lhsT=ones3[:],
                        rhs=B4sq[:, q * 512 : (q + 1) * 512],
                        start=True,
                        stop=True,
                    )
                    nc.scalar.activation(
                        B4[0:1, q * 512 : (q + 1) * 512], qp[:], CPY, scale=-0.5
                    )
                nc.sync.dma_start(B4[1:4, :], C3[:])
                A4 = persist.tile([4, NCK], f32, tag=f"A4_{g}")
                nc.sync.dma_start(A4[0:1, :], ones_d[:])
                nc.sync.dma_start(A4[1:4, :], C3[:])
                B4s.append(B4)
                A4s.append(A4)

        # ============ Phase D: KNN (select + refine)  |  Phase E: linear ======
        with tc.tile_pool(name="d2psum", bufs=2, space="PSUM") as d2psum, \
             tc.tile_pool(name="linpsum", bufs=2, space="PSUM") as linpsum, \
             tc.tile_pool(name="sd2p", bufs=2) as sd2p, \
             tc.tile_pool(name="small", bufs=3) as small, \
             tc.tile_pool(name="linp", bufs=3) as linp:
            for g in range(GPC):
                A4, B4, pk = A4s[g], B4s[g], pks[g]
                R = persist.tile([128, 256], f32, tag=f"R_{g}")
                RI = persist.tile([128, 256], f32, tag=f"RI_{g}")
                for rcg in range(8):  # groups of 4 rank-chunks
                    G4 = small.tile([128, 96], f32, tag="G4")
                    M4 = small.tile([128, 32], i32, tag="M4")
                    for rc4 in range(4):
                        rc = rcg * 4 + rc4
                        sd2 = sd2p.tile([128, NCK], f32)
                        for quar in range(4):
                            dp = d2psum.tile([128, 1024], f32)
                            for q in range(2):
                                c0 = quar * 1024 + q * 512
                                nc.tensor.matmul(
                                    out=dp[:, q * 512 : (q + 1) * 512],
                                    lhsT=A4[:, rc * 128 : (rc + 1) * 128],
                                    rhs=B4[:, c0 : c0 + 512],
                                    start=True,
                                    stop=True,
                                )
                            nc.scalar.activation(
                                sd2[:, quar * 1024 : (quar + 1) * 1024], dp[:], CPY
                            )
                        if "scan" in _SKIP:
                            continue
                        mx = small.tile([128, 8], f32, tag="mx")
                        nc.vector.max(mx[:], sd2[:])
                        mi = small.tile([128, 8], u32, tag="mi")
                        nc.vector.max_index(mi[:], mx[:], sd2[:])
                        nc.vector.tensor_copy(
                            M4[:, rc4 * 8 : (rc4 + 1) * 8], mi[:]
                        )
                        nc.vector.tensor_copy(
                            RI[:, rc * 8 : (rc + 1) * 8], mi[:]
                        )
                    if _SKIP & {"scan", "gather"}:
                        continue
                    # candidate coord gathers: one row-gather per candidate slot
                    for j in range(32):
                        nc.gpsimd.indirect_dma_start(
                            out=G4[:, j * 3 : (j + 1) * 3],
                            out_offset=None,
                            in_=pselflat,
                            in_offset=bass.IndirectOffsetOnAxis(
                                ap=M4[:, j : j + 1], axis=0
                            ),
                            element_offset=g * NCK * 3,
                        )
                    if "refine" in _SKIP:
                        continue
                    # exact d2, same fp32 association order as the reference
                    Gv = G4[:].rearrange("p (j c) -> p c j", c=3)
                    rsl = R[:, rcg * 32 : (rcg + 1) * 32]
                    sqs = []
                    pkv = pk[:].rearrange("p (rc c) -> p rc c", c=3)
                    for c in range(3):
                        pkb = pkv[
                            :, rcg * 4 : (rcg + 1) * 4, c : c + 1
                        ].to_broadcast([128, 4, 8])
                        dc = small.tile([128, 32], f32, tag=f"dc{c}")
                        nc.vector.tensor_tensor(
                            dc[:].rearrange("p (rc j) -> p rc j", j=8),
                            Gv[:, c, :].rearrange("p (rc j) -> p rc j", j=8),
                            pkb,
                            op=Alu.subtract,
                        )
                        sq = small.tile([128, 32], f32, tag=f"sq{c}")
                        nc.vector.tensor_tensor(sq[:], dc[:], dc[:], op=Alu.mult)
                        sqs.append(sq)
                    s01 = small.tile([128, 32], f32, tag="s01")
                    nc.vector.tensor_tensor(s01[:], sqs[0][:], sqs[1][:], op=Alu.add)
                    nc.vector.tensor_tensor(rsl, s01[:], sqs[2][:], op=Alu.add)

                if _SKIP & {"scan", "gather", "refine", "sort8"}:
                    if "E" in _SKIP:
                        continue
                    for rc in range(32):
                        r0 = g * NCK + rc * 128
                        xs = linp.tile([128, 128], f32, tag="xs")
                        nc.sync.dma_start(xs[:], xsel[r0 : r0 + 128, :])
                        xtp2 = tpsum.tile([128, 128], f32, tag="xtp")
                        nc.tensor.transpose(
                            out=xtp2[:], in_=xs[:], identity=ident[:]
                        )
                        xts2 = linp.tile([128, 128], f32, tag="xts2")
                        nc.scalar.activation(xts2[:], xtp2[:], CPY)
                        lp = linpsum.tile([128, 128], f32)
                        nc.tensor.matmul(
                            out=lp[:], lhsT=xts2[:], rhs=wlt_sb[:],
                            start=True, stop=False,
                        )
                        nc.tensor.matmul(
                            out=lp[:], lhsT=ones1[:], rhs=bb_sb[:],
                            start=False, stop=True,
                        )
                        xo = linp.tile([128, 128], f32, tag="xo")
                        nc.scalar.activation(xo[:], lp[:], CPY)
                        nc.sync.dma_start(xc_out[r0 : r0 + 128, :], xo[:])
                    continue
                # graph-wide self-candidate masking
                msw = scratch.tile([128, 256], i32, tag="mi32")
                nc.vector.tensor_tensor(msw[:], RI[:], rankof_sb[:], op=Alu.is_equal)
                nc.vector.copy_predicated(R[:], msw[:], bigw[:])

                # stable sort-of-8 segments, ascending by (d2, idx)
                def seg8(ap, s):
                    v = ap.rearrange(
                        "p (seg blk t e) -> p seg blk t e", seg=32, t=2, e=s
                    )
                    return v[:, :, :, 0, :], v[:, :, :, 1, :]

                for kk8 in range(1, 4):
                    for jj8 in range(kk8 - 1, -1, -1):
                        s = 1 << jj8
                        av, bv = seg8(R[:], s)
                        ai, bi = seg8(RI[:], s)
                        gtv = scratch.tile([128, 256], f32, tag="lt")
                        eqv = scratch.tile([128, 256], f32, tag="m")
                        gti = scratch.tile([128, 256], f32, tag="tv")
                        m8 = scratch.tile([128, 256], i32, tag="mi32")
                        t8v = scratch.tile([128, 256], f32, tag="t8v")
                        t8i = scratch.tile([128, 256], f32, tag="t8i")
                        gtvv = seg8(gtv[:], s)[0]
                        eqvv = seg8(eqv[:], s)[0]
                        gtiv = seg8(gti[:], s)[0]
                        m8v = seg8(m8[:], s)[0]
                        t8vv = seg8(t8v[:], s)[0]
                        t8iv = seg8(t8i[:], s)[0]
                        nc.vector.tensor_tensor(gtvv, av, bv, op=Alu.is_gt)
                        nc.vector.tensor_tensor(eqvv, av, bv, op=Alu.is_equal)
                        nc.vector.tensor_tensor(gtiv, ai, bi, op=Alu.is_gt)
                        nc.vector.tensor_tensor(eqvv, eqvv, gtiv, op=Alu.mult)
                        nc.vector.tensor_tensor(gtvv, gtvv, eqvv, op=Alu.add)
                        krow8 = kk8 if kk8 < 3 else 7  # row 7 all-zero (asc)
                        fbv = seg8(fb_sb[:, krow8 * 256 : (krow8 + 1) * 256], s)[0]
                        nc.vector.tensor_tensor(m8v, gtvv, fbv, op=Alu.subtract)
                        nc.vector.tensor_copy(t8vv, av)
                        nc.vector.copy_predicated(av, m8v, bv)
                        nc.vector.copy_predicated(bv, m8v, t8vv)
                        nc.vector.tensor_copy(t8iv, ai)
                        nc.vector.copy_predicated(ai, m8v, bi)
                        nc.vector.copy_predicated(bi, m8v, t8iv)

                RIi = persist.tile([128, 192], i32, tag=f"RIi_{g}")
                nc.vector.tensor_copy(
                    RIi[:].rearrange("p (seg k) -> p seg k", k=6),
                    RI[:].rearrange("p (seg k) -> p seg k", k=8)[:, :, 0:6],
                )
                nc.sync.dma_start(
                    nbr_out[g].rearrange("(rc p) k -> p rc k", p=128),
                    RIi[:].rearrange("p (rc k) -> p rc k", k=6),
                )

                # -------- Phase E: linear on the pre-gathered rows --------
                if "E" in _SKIP:
                    continue
                for rc in range(32):
                    r0 = g * NCK + rc * 128
                    xs = linp.tile([128, 128], f32, tag="xs")
                    nc.sync.dma_start(xs[:], xsel[r0 : r0 + 128, :])
                    xtp2 = tpsum.tile([128, 128], f32, tag="xtp")
                    nc.tensor.transpose(out=xtp2[:], in_=xs[:], identity=ident[:])
                    xts2 = linp.tile([128, 128], f32, tag="xts2")
                    nc.scalar.activation(xts2[:], xtp2[:], CPY)
                    lp = linpsum.tile([128, 128], f32)
                    nc.tensor.matmul(
                        out=lp[:], lhsT=xts2[:], rhs=wlt_sb[:], start=True, stop=False
                    )
                    nc.tensor.matmul(
                        out=lp[:], lhsT=ones1[:], rhs=bb_sb[:], start=False, stop=True
                    )
                    xo = linp.tile([128, 128], f32, tag="xo")
                    nc.scalar.activation(xo[:], lp[:], CPY)
                    nc.sync.dma_start(xc_out[r0 : r0 + 128, :], xo[:])

    return nc


def _get_program():
    if "nc" not in _cache:
        nc = _build_program()
        if not nc.is_finalized():
            nc.finalize()  # run bacc passes (reg alloc, wait splitting)
        _cache["nc"] = nc
    return _cache["nc"]


def _select_topk(x, w_sel):
    """Replicate the reference's selection pipeline bit-exactly.

    Mirrors: score = (x @ w_sel) / jnp.linalg.norm(w_sel)  (numpy matmul on
    host when inputs are numpy, device otherwise — exactly as the reference
    receives them), softmax on device, and top-k with lowest-index
    tie-breaking (jax.lax.top_k ≡ stable argsort on the softmax values).
    Returns idx [B, NCK] int32.
    """
    import jax
    import jax.numpy as jnp

    score = (x @ w_sel) / jnp.linalg.norm(w_sel)
    v = jax.nn.softmax(jnp.asarray(score).reshape(B, N), axis=-1)
    vn = np.asarray(v)
    idx = np.argsort(-vn, axis=1, kind="stable")[:, :NCK]
    return np.ascontiguousarray(idx.astype(np.int32))


def _make_in_maps(x, pos, idx, w_lin, b_lin):
    x = np.asarray(x, dtype=np.float32)
    pos = np.asarray(pos, dtype=np.float32)
    perm = (
        idx.astype(np.int64) + (np.arange(B, dtype=np.int64) * N)[:, None]
    ).reshape(-1)
    xsel = np.ascontiguousarray(x[perm])  # [B*NCK, D]
    psel = np.ascontiguousarray(pos[perm]).reshape(B, NCK, 3)
    wlt = np.ascontiguousarray(np.asarray(w_lin, dtype=np.float32).T)
    bb = np.ascontiguousarray(np.asarray(b_lin, dtype=np.float32).reshape(1, D))
    in_maps = []
    for c in range(M):
        in_maps.append(
            {
                "xsel": xsel[c * GPC * NCK : (c + 1) * GPC * NCK],
                "psel": psel[c * GPC : (c + 1) * GPC],
                "wlt": wlt,
                "bb": bb,
            }
        )
    return in_maps, perm


def _assemble(results, idx, perm, batch):
    x_c = np.concatenate([r["xc_out"] for r in results], axis=0)  # [B*NCK, D]
    pos_c = np.concatenate(
        [r["pc_out"].reshape(GPC * NCK, 3) for r in results], axis=0
    )
    nbr = np.concatenate([r["nbr_out"] for r in results], axis=0)  # [B, NCK, KNN]

    batch_c = np.asarray(batch)[perm]

    off = (np.arange(B, dtype=np.int32) * NCK)[:, None, None]
    src = (nbr.astype(np.int32) + off).reshape(-1)
    ctr = np.broadcast_to(
        np.arange(NCK, dtype=np.int32)[None, :, None] + off, (B, NCK, KNN)
    ).reshape(-1)
    edge_index = np.stack(
        [np.concatenate([src, ctr]), np.concatenate([ctr, src])]
    ).astype(np.int32)
    return x_c, pos_c, batch_c, edge_index


def kernel(x, pos, batch, w_sel, w_lin, b_lin):
    from concourse.bass_utils import run_bass_kernel_spmd

    nc = _get_program()
    idx = _select_topk(x, w_sel)
    in_maps, perm = _make_in_maps(x, pos, idx, w_lin, b_lin)
    res = run_bass_kernel_spmd(nc, in_maps, list(range(M)))
    return _assemble(res.results, idx, perm, batch)
